# revision 54
# baseline (speedup 1.0000x reference)
"""ASPP + pixel-shuffle upsample + 1x1 project, on 8 TRN2 NeuronCores.

Strategy: data-parallel over batch (B=8 -> 1 image per core). Per core:
  - all convs as matmuls on the PE (bf16 inputs/weights, fp32 PSUM accum)
  - BN folded into conv weights/bias on host
  - 3x3 dilated convs = 9 shifted 1x1 taps accumulated in PSUM; each tap
    computes only its valid (non-zero-padding) region. PSUM spatial chunks
    are laid out column-major so a tap's column restriction is a contiguous
    PSUM range; x is stored row-major with 18 zero rows of top/bottom pad
    (row-shifted taps read zero rows; fully-zero chunks are skipped).
  - interleave (pixel-shuffle) is never materialized: the 1x1 projection is
    applied per-branch and its ReLU output is written with a strided AP
    directly into the interleaved position of the output row buffer
  - output rows stream back to DRAM per 16-row block
"""

import numpy as np
import ml_dtypes

B, CIN, COUT, H = 8, 256, 128, 64
PAD = 18
XR = H + 2 * PAD  # padded rows: 100
EPS = 1e-5
RATES = (6, 12, 18)
N_CORES = 8
NTAP = 28  # 1 (branch0 1x1) + 3 branches * 9 taps

_BF16 = ml_dtypes.bfloat16


def _branch_taps(t):
    """[(weight_block, sy, sx)] for branch t, center tap first."""
    if t == 0:
        return [(0, 0, 0)]
    d = RATES[t - 1]
    base = 1 + 9 * (t - 1)
    taps = []
    for ky in range(3):
        for kx in range(3):
            taps.append((base + ky * 3 + kx, (ky - 1) * d, (kx - 1) * d))
    taps.sort(key=lambda w: (w[1] != 0 or w[2] != 0))  # center first
    return taps


def build_program(edge_trim=True):
    # edge_trim=False keeps every matmul's PSUM write 2-D so CoreSim can
    # check it; True additionally trims zero-pad rows at sy-edge chunks
    # (3-D PSUM writes the simulator can't shape-check — validated on HW)
    import concourse.mybir as mybir
    import concourse.tile as tile
    from concourse import bacc

    f32, bf16 = mybir.dt.float32, mybir.dt.bfloat16
    Relu = mybir.ActivationFunctionType.Relu
    Alu = mybir.AluOpType

    # with edge_trim every matmul reads only real input rows, so x ships
    # without the 18-row conv pads; rows are padded to 66 anyway so the
    # per-column stride (132B) isn't a power of two (a 128B stride makes
    # the PE's strided rhs reads bank-conflict). The sim variant
    # (edge_trim=False) reads zero-pad rows and needs the full padded layout.
    xr = 66 if edge_trim else XR
    pad = 0 if edge_trim else PAD
    nc = bacc.Bacc("TRN2", target_bir_lowering=False, debug=False)
    xp = nc.dram_tensor("xp", [2, 128, H * xr], bf16, kind="ExternalInput")
    wb = nc.dram_tensor("wb", [2, 128, NTAP * 128], bf16, kind="ExternalInput")
    wp = nc.dram_tensor("wp", [128, 128], bf16, kind="ExternalInput")
    bias = nc.dram_tensor("bias", [128, 5], f32, kind="ExternalInput")
    out = nc.dram_tensor("out", [128, 4 * H * H], f32, kind="ExternalOutput")

    with tile.TileContext(nc) as tc:
        with (
            tc.tile_pool(name="const", bufs=1) as cpool,
            tc.tile_pool(name="bf", bufs=3) as bfpool,
            tc.tile_pool(name="ob", bufs=3) as opool,
            tc.tile_pool(name="psA", bufs=3, space="PSUM") as psA,
            tc.tile_pool(name="psB", bufs=3, space="PSUM") as psB,
        ):
            # PE warm-up: dummy matmuls on a zeroed scratch tile release the
            # HAM clock throttle while the input DMAs are still in flight
            scratch = cpool.tile([128, 512], bf16, tag="scratch")
            nc.vector.memset(scratch[:], 0.0)
            psW = psA.tile([128, 512], f32, tag="warm", bufs=1)
            for i in range(16):
                nc.tensor.matmul(
                    psW[:], lhsT=scratch[:, :128], rhs=scratch[:],
                    start=(i == 0), stop=(i == 15), skip_group_check=True,
                )
            bt = cpool.tile([128, 5], f32, tag="bias")
            nc.sync.dma_start(out=bt, in_=bias[:])
            wpt = cpool.tile([128, 128], bf16, tag="wp")
            nc.sync.dma_start(out=wpt, in_=wp[:])
            # x stored column-major: [128, 64 cols x 100 rows], rows 18..82
            # hold the image (transposed + row-padded on host), so the DMA is
            # fully contiguous and matmul rhs APs have 8 contiguous rows
            # innermost. Issue order: x chunk 0, then the weights the first
            # chunk's branches need, then x chunk 1, then branch-3 weights.
            wt = [
                cpool.tile([128, NTAP * 128], bf16, tag=f"w{c}", name=f"w{c}")
                for c in range(2)
            ]
            xtile = [
                cpool.tile([128, H * xr], bf16, tag=f"x{c}", name=f"x{c}")
                for c in range(2)
            ]
            x3t = [
                xtile[c].rearrange("p (w h) -> p w h", h=xr) for c in range(2)
            ]
            # few, big, mostly-contiguous input DMAs that finish ASAP: matmuls
            # overlapped with in-flight input DMA measure ~20% slower, so
            # stretching the input phase costs more than it hides
            nc.sync.dma_start(out=xtile[0], in_=xp[0])
            nc.sync.dma_start(out=wt[0][:, : 19 * 128], in_=wb[0][:, : 19 * 128])
            nc.sync.dma_start(out=xtile[1], in_=xp[1])
            nc.sync.dma_start(out=wt[1][:, : 19 * 128], in_=wb[1][:, : 19 * 128])
            for c in range(2):  # branch 3 weights last
                nc.sync.dma_start(out=wt[c][:, 19 * 128 :], in_=wb[c][:, 19 * 128 :])

            out3 = out.rearrange("p (a b) -> p a b", b=2 * H)

            def emit_group(ps, k, t, c_list, is_start, is_stop, skip_check=False):
                """Emit the conv matmuls of branch t, chunk k, for the given
                cin-chunks, accumulating into psum tile ps."""
                mms = []
                for c in c_list:
                    for blk, sy, sx in _branch_taps(t):
                        if 8 * k + 8 + sy <= 0 or 8 * k + sy >= H:
                            continue  # all rows read zero-pad: contributes 0
                        c0, c1 = max(0, -sx), min(H, H - sx)
                        mms.append((blk, sy, sx, c0, c1, c))
                n = len(mms)
                ps3 = ps.rearrange("p (w h) -> p w h", h=8)
                for idx, (blk, sy, sx, c0, c1, c) in enumerate(mms):
                    r0 = pad + 8 * k + sy
                    # rows of this chunk whose input row is real data
                    # (the rest read zero pad: contribute nothing)
                    a0 = max(0, -sy - 8 * k) if edge_trim else 0
                    a1 = min(8, H - sy - 8 * k) if edge_trim else 8
                    first = is_start and idx == 0
                    last = is_stop and idx == n - 1
                    if (a0, a1) != (0, 8) and not first:
                        rhs = x3t[c][:, c0 + sx : c1 + sx, r0 + a0 : r0 + a1]
                        dst = ps3[:, c0:c1, a0:a1]
                    elif (c1 - c0) == H:
                        rhs = x3t[c][:, c0 + sx : c1 + sx, r0 : r0 + 8]
                        dst = ps[:]
                    else:
                        rhs = x3t[c][:, c0 + sx : c1 + sx, r0 : r0 + 8]
                        dst = ps[:, c0 * 8 : c1 * 8]
                    nc.tensor.matmul(
                        dst,
                        lhsT=wt[c][:, blk * 128 : (blk + 1) * 128],
                        rhs=rhs,
                        start=first,
                        stop=last,
                        skip_group_check=skip_check,
                    )

            for k in range(8):  # 8-row input chunks -> output rows 16k..16k+16
                ob = opool.tile([128, 16 * 2 * H], f32, tag="ob")
                ob3 = ob.rearrange("p (a b) -> p a b", b=2 * H)
                # (out-col, out-row) view matching the col-major psum layout
                obt = ob3.rearrange("p a b -> p b a")
                done = set()
                # k=0: branches with long cin-chunk-0 prefixes first, so the
                # PE has work before x chunk 1 lands
                for t in ([1, 2, 3, 0] if k == 0 else range(4)):
                    ps = psA.tile([128, 512], f32, tag="ps")
                    emit_group(ps, k, t, [0, 1], True, True)
                    bftile = bfpool.tile([128, 512], bf16, tag="bf")
                    nc.scalar.activation(bftile[:], ps[:], Relu, bias=bt[:, t : t + 1])
                    ps2 = psB.tile([128, 512], f32, tag="ps2")
                    nc.tensor.matmul(
                        ps2[:], lhsT=wpt[:], rhs=bftile[:], start=True, stop=True
                    )
                    r_, c_ = t // 2, t % 2
                    ps2v = ps2.rearrange("p (w h) -> p w h", h=8)
                    if k < 7:
                        # projection relu on the Vector engine: keeps the
                        # Scalar engine free for the branch relus (halves the
                        # per-chunk relu chain)
                        nc.vector.tensor_scalar(
                            obt[:, c_::2, r_::2], ps2v,
                            bt[:, 4:5], 0.0, op0=Alu.add, op1=Alu.max,
                        )
                        done.add(t)
                        # stream each output-row parity out as soon as the
                        # two branches feeding it are done
                        if done >= {0, 1} and "even" not in done:
                            nc.sync.dma_start(
                                out=out3[:, 16 * k : 16 * (k + 1) : 2, :],
                                in_=ob3[:, 0::2, :],
                            )
                            done.add("even")
                        if done >= {2, 3} and "odd" not in done:
                            nc.sync.dma_start(
                                out=out3[:, 16 * k + 1 : 16 * (k + 1) : 2, :],
                                in_=ob3[:, 1::2, :],
                            )
                            done.add("odd")
                    else:
                        # last chunk: write row-quarters so the final DMAs are
                        # contiguous and the very last one is small
                        for h in range(4):
                            eng = nc.vector if h % 2 == 0 else nc.scalar
                            dst_q = obt[:, c_::2, r_ + 4 * h : 4 + 4 * h : 2]
                            src_q = ps2v[:, :, 2 * h : 2 * (h + 1)]
                            if h % 2 == 0:
                                nc.vector.tensor_scalar(
                                    dst_q, src_q, bt[:, 4:5], 0.0,
                                    op0=Alu.add, op1=Alu.max,
                                )
                            else:
                                nc.scalar.activation(
                                    dst_q, src_q, Relu, bias=bt[:, 4:5]
                                )
                        done.add(t)
                        if done >= {0, 1, 2, 3}:
                            for h in range(4):
                                nc.sync.dma_start(
                                    out=out3[:, 16 * k + 4 * h : 16 * k + 4 * (h + 1), :],
                                    in_=ob3[:, 4 * h : 4 * (h + 1), :],
                                )
    nc.compile()
    return nc


def host_prep_weights(inputs):
    f32 = np.float32
    scales, biases = [], []
    for t in ("0", "1", "2", "3", "p"):
        g = np.asarray(inputs[f"g{t}"], f32)
        b = np.asarray(inputs[f"b{t}"], f32)
        m = np.asarray(inputs[f"m{t}"], f32)
        v = np.asarray(inputs[f"v{t}"], f32)
        s = g / np.sqrt(v + EPS)
        scales.append(s)
        biases.append((b - m * s).astype(f32))
    bias_arr = np.stack(biases, axis=1).astype(f32)  # (128, 5)

    wtaps = np.zeros((NTAP, CIN, COUT), f32)  # [tap, ci, co]
    w0 = np.asarray(inputs["w0"], f32)[:, :, 0, 0] * scales[0][:, None]  # (co, ci)
    wtaps[0] = w0.T
    blk = 1
    for bi, key in enumerate(("w1", "w2", "w3")):
        w = np.asarray(inputs[key], f32) * scales[bi + 1][:, None, None, None]
        for ky in range(3):
            for kx in range(3):
                wtaps[blk] = w[:, :, ky, kx].T
                blk += 1
    wb = (
        wtaps.reshape(NTAP, 2, 128, COUT)
        .transpose(1, 2, 0, 3)
        .reshape(2, 128, NTAP * COUT)
        .astype(_BF16)
    )
    wpT = (
        (np.asarray(inputs["wp"], f32)[:, :, 0, 0] * scales[4][:, None])
        .T.astype(_BF16)
        .copy()
    )
    return wb, wpT, bias_arr


def host_prep_x(x, padded=False):
    # transpose each image to (col, row) matching the device's column-major
    # SBUF layout, so the device DMA is one contiguous copy per cin-chunk.
    # padded=True bakes the 18-row zero pad (sim variant only).
    x = np.asarray(x, np.float32).reshape(B, 2, 128, H, H)
    if padded:
        xt = np.zeros((B, 2, 128, H, XR), np.float32)
        xt[:, :, :, :, PAD : PAD + H] = x.transpose(0, 1, 2, 4, 3)
        return xt.reshape(B, 2, 128, H * XR).astype(_BF16)
    xt = np.zeros((B, 2, 128, H, 66), np.float32)
    xt[:, :, :, :, :H] = x.transpose(0, 1, 2, 4, 3)
    return xt.reshape(B, 2, 128, H * 66).astype(_BF16)


def make_in_maps(inputs, padded=False):
    wb, wpT, bias_arr = host_prep_weights(inputs)
    xq = host_prep_x(inputs["x"], padded=padded)
    return [{"xp": xq[b], "wb": wb, "wp": wpT, "bias": bias_arr} for b in range(B)]


_NC_CACHE = []


def kernel(**inputs):
    from concourse import bass_utils

    if not _NC_CACHE:
        _NC_CACHE.append(build_program())
    nc = _NC_CACHE[0]
    in_maps = make_in_maps(inputs)
    res = bass_utils.run_bass_kernel_spmd(nc, in_maps, core_ids=list(range(N_CORES)))
    return np.stack(
        [r["out"].reshape(COUT, 2 * H, 2 * H) for r in res.results]
    ).astype(np.float32)


# revision 56
# speedup vs baseline: 1.2054x; 1.2054x over previous
"""ASPP + pixel-shuffle upsample + 1x1 project, on 8 TRN2 NeuronCores.

Strategy: data-parallel over batch (B=8 -> 1 image per core). Per core:
  - all convs as matmuls on the PE (bf16 inputs/weights, fp32 PSUM accum)
  - BN folded into conv weights/bias on host
  - 3x3 dilated convs = 9 shifted 1x1 taps accumulated in PSUM; each tap
    computes only its valid (non-zero-padding) region. PSUM spatial chunks
    are laid out column-major so a tap's column restriction is a contiguous
    PSUM range; x is stored row-major with 18 zero rows of top/bottom pad
    (row-shifted taps read zero rows; fully-zero chunks are skipped).
  - interleave (pixel-shuffle) is never materialized: the 1x1 projection is
    applied per-branch and its ReLU output is written with a strided AP
    directly into the interleaved position of the output row buffer
  - output rows stream back to DRAM per 16-row block
"""

import numpy as np
import ml_dtypes

B, CIN, COUT, H = 8, 256, 128, 64
PAD = 18
XR = H + 2 * PAD  # padded rows: 100
EPS = 1e-5
RATES = (6, 12, 18)
N_CORES = 8
NTAP = 28  # 1 (branch0 1x1) + 3 branches * 9 taps

_BF16 = ml_dtypes.bfloat16


def _branch_taps(t):
    """[(weight_block, sy, sx)] for branch t, center tap first."""
    if t == 0:
        return [(0, 0, 0)]
    d = RATES[t - 1]
    base = 1 + 9 * (t - 1)
    taps = []
    for ky in range(3):
        for kx in range(3):
            taps.append((base + ky * 3 + kx, (ky - 1) * d, (kx - 1) * d))
    taps.sort(key=lambda w: (w[1] != 0 or w[2] != 0))  # center first
    return taps


def build_program(edge_trim=True):
    # edge_trim=False keeps every matmul's PSUM write 2-D so CoreSim can
    # check it; True additionally trims zero-pad rows at sy-edge chunks
    # (3-D PSUM writes the simulator can't shape-check — validated on HW)
    import concourse.mybir as mybir
    import concourse.tile as tile
    from concourse import bacc

    f32, bf16 = mybir.dt.float32, mybir.dt.bfloat16
    Relu = mybir.ActivationFunctionType.Relu
    Alu = mybir.AluOpType

    # with edge_trim every matmul reads only real input rows, so x ships
    # without the 18-row conv pads; rows are padded to 66 anyway so the
    # per-column stride (132B) isn't a power of two (a 128B stride makes
    # the PE's strided rhs reads bank-conflict). The sim variant
    # (edge_trim=False) reads zero-pad rows and needs the full padded layout.
    xr = 66 if edge_trim else XR
    pad = 0 if edge_trim else PAD
    nc = bacc.Bacc("TRN2", target_bir_lowering=False, debug=False)
    xp = nc.dram_tensor("xp", [2, 128, H * xr], bf16, kind="ExternalInput")
    wb = nc.dram_tensor("wb", [2, 128, NTAP * 128], bf16, kind="ExternalInput")
    wp = nc.dram_tensor("wp", [128, 128], bf16, kind="ExternalInput")
    bias = nc.dram_tensor("bias", [128, 5], f32, kind="ExternalInput")
    out = nc.dram_tensor("out", [128, 4 * H * H], f32, kind="ExternalOutput")

    with tile.TileContext(nc) as tc:
        with (
            tc.tile_pool(name="const", bufs=1) as cpool,
            tc.tile_pool(name="bf", bufs=3) as bfpool,
            tc.tile_pool(name="ob", bufs=3) as opool,
            tc.tile_pool(name="psA", bufs=3, space="PSUM") as psA,
            tc.tile_pool(name="psB", bufs=3, space="PSUM") as psB,
        ):
            # PE warm-up: dummy matmuls on a zeroed scratch tile release the
            # HAM clock throttle while the input DMAs are still in flight
            scratch = cpool.tile([128, 512], bf16, tag="scratch")
            nc.vector.memset(scratch[:], 0.0)
            psW = psA.tile([128, 512], f32, tag="warm", bufs=1)
            for i in range(16):
                nc.tensor.matmul(
                    psW[:], lhsT=scratch[:, :128], rhs=scratch[:],
                    start=(i == 0), stop=(i == 15), skip_group_check=True,
                )
            bt = cpool.tile([128, 5], f32, tag="bias")
            nc.sync.dma_start(out=bt, in_=bias[:])
            wpt = cpool.tile([128, 128], bf16, tag="wp")
            nc.sync.dma_start(out=wpt, in_=wp[:])
            # x stored column-major: [128, 64 cols x 100 rows], rows 18..82
            # hold the image (transposed + row-padded on host), so the DMA is
            # fully contiguous and matmul rhs APs have 8 contiguous rows
            # innermost. Issue order: x chunk 0, then the weights the first
            # chunk's branches need, then x chunk 1, then branch-3 weights.
            wt = [
                cpool.tile([128, NTAP * 128], bf16, tag=f"w{c}", name=f"w{c}")
                for c in range(2)
            ]
            xtile = [
                cpool.tile([128, H * xr], bf16, tag=f"x{c}", name=f"x{c}")
                for c in range(2)
            ]
            x3t = [
                xtile[c].rearrange("p (w h) -> p w h", h=xr) for c in range(2)
            ]
            # few, big, mostly-contiguous input DMAs that finish ASAP: matmuls
            # overlapped with in-flight input DMA measure ~20% slower, so
            # stretching the input phase costs more than it hides
            nc.sync.dma_start(out=xtile[0], in_=xp[0])
            nc.sync.dma_start(out=wt[0][:, : 19 * 128], in_=wb[0][:, : 19 * 128])
            nc.sync.dma_start(out=xtile[1], in_=xp[1])
            nc.sync.dma_start(out=wt[1][:, : 19 * 128], in_=wb[1][:, : 19 * 128])
            for c in range(2):  # branch 3 weights last
                nc.sync.dma_start(out=wt[c][:, 19 * 128 :], in_=wb[c][:, 19 * 128 :])

            out3 = out.rearrange("p (a b) -> p a b", b=2 * H)

            def emit_group(ps, k, t, c_list, is_start, is_stop, skip_check=False):
                """Emit the conv matmuls of branch t, chunk k, for the given
                cin-chunks, accumulating into psum tile ps."""
                mms = []
                for c in c_list:
                    for blk, sy, sx in _branch_taps(t):
                        if 8 * k + 8 + sy <= 0 or 8 * k + sy >= H:
                            continue  # all rows read zero-pad: contributes 0
                        c0, c1 = max(0, -sx), min(H, H - sx)
                        mms.append((blk, sy, sx, c0, c1, c))
                n = len(mms)
                ps3 = ps.rearrange("p (w h) -> p w h", h=8)
                for idx, (blk, sy, sx, c0, c1, c) in enumerate(mms):
                    r0 = pad + 8 * k + sy
                    # rows of this chunk whose input row is real data
                    # (the rest read zero pad: contribute nothing)
                    a0 = max(0, -sy - 8 * k) if edge_trim else 0
                    a1 = min(8, H - sy - 8 * k) if edge_trim else 8
                    first = is_start and idx == 0
                    last = is_stop and idx == n - 1
                    if (a0, a1) != (0, 8) and not first:
                        rhs = x3t[c][:, c0 + sx : c1 + sx, r0 + a0 : r0 + a1]
                        dst = ps3[:, c0:c1, a0:a1]
                    elif (c1 - c0) == H:
                        rhs = x3t[c][:, c0 + sx : c1 + sx, r0 : r0 + 8]
                        dst = ps[:]
                    else:
                        rhs = x3t[c][:, c0 + sx : c1 + sx, r0 : r0 + 8]
                        dst = ps[:, c0 * 8 : c1 * 8]
                    nc.tensor.matmul(
                        dst,
                        lhsT=wt[c][:, blk * 128 : (blk + 1) * 128],
                        rhs=rhs,
                        start=first,
                        stop=last,
                        skip_group_check=skip_check,
                    )

            for k in range(8):  # 8-row input chunks -> output rows 16k..16k+16
                ob = opool.tile([128, 16 * 2 * H], f32, tag="ob")
                ob3 = ob.rearrange("p (a b) -> p a b", b=2 * H)
                # (out-col, out-row) view matching the col-major psum layout
                obt = ob3.rearrange("p a b -> p b a")
                done = set()
                # k=0: branches with long cin-chunk-0 prefixes first, so the
                # PE has work before x chunk 1 lands
                for t in ([1, 2, 3, 0] if k == 0 else range(4)):
                    ps = psA.tile([128, 512], f32, tag="ps")
                    emit_group(ps, k, t, [0, 1], True, True)
                    bftile = bfpool.tile([128, 512], bf16, tag="bf")
                    nc.scalar.activation(bftile[:], ps[:], Relu, bias=bt[:, t : t + 1])
                    ps2 = psB.tile([128, 512], f32, tag="ps2")
                    nc.tensor.matmul(
                        ps2[:], lhsT=wpt[:], rhs=bftile[:], start=True, stop=True
                    )
                    r_, c_ = t // 2, t % 2
                    ps2v = ps2.rearrange("p (w h) -> p w h", h=8)
                    if k < 7:
                        nc.scalar.activation(
                            obt[:, c_::2, r_::2], ps2v, Relu, bias=bt[:, 4:5]
                        )
                        done.add(t)
                        # stream each output-row parity out as soon as the
                        # two branches feeding it are done
                        if done >= {0, 1} and "even" not in done:
                            nc.sync.dma_start(
                                out=out3[:, 16 * k : 16 * (k + 1) : 2, :],
                                in_=ob3[:, 0::2, :],
                            )
                            done.add("even")
                        if done >= {2, 3} and "odd" not in done:
                            nc.sync.dma_start(
                                out=out3[:, 16 * k + 1 : 16 * (k + 1) : 2, :],
                                in_=ob3[:, 1::2, :],
                            )
                            done.add("odd")
                    else:
                        # last chunk: write row-quarters so the final DMAs are
                        # contiguous and the very last one is small
                        for h in range(4):
                            nc.scalar.activation(
                                obt[:, c_::2, r_ + 4 * h : 4 + 4 * h : 2],
                                ps2v[:, :, 2 * h : 2 * (h + 1)],
                                Relu,
                                bias=bt[:, 4:5],
                            )
                        done.add(t)
                        if done >= {0, 1, 2, 3}:
                            for h in range(4):
                                nc.sync.dma_start(
                                    out=out3[:, 16 * k + 4 * h : 16 * k + 4 * (h + 1), :],
                                    in_=ob3[:, 4 * h : 4 * (h + 1), :],
                                )
    nc.compile()
    return nc


def host_prep_weights(inputs):
    f32 = np.float32
    scales, biases = [], []
    for t in ("0", "1", "2", "3", "p"):
        g = np.asarray(inputs[f"g{t}"], f32)
        b = np.asarray(inputs[f"b{t}"], f32)
        m = np.asarray(inputs[f"m{t}"], f32)
        v = np.asarray(inputs[f"v{t}"], f32)
        s = g / np.sqrt(v + EPS)
        scales.append(s)
        biases.append((b - m * s).astype(f32))
    bias_arr = np.stack(biases, axis=1).astype(f32)  # (128, 5)

    wtaps = np.zeros((NTAP, CIN, COUT), f32)  # [tap, ci, co]
    w0 = np.asarray(inputs["w0"], f32)[:, :, 0, 0] * scales[0][:, None]  # (co, ci)
    wtaps[0] = w0.T
    blk = 1
    for bi, key in enumerate(("w1", "w2", "w3")):
        w = np.asarray(inputs[key], f32) * scales[bi + 1][:, None, None, None]
        for ky in range(3):
            for kx in range(3):
                wtaps[blk] = w[:, :, ky, kx].T
                blk += 1
    wb = (
        wtaps.reshape(NTAP, 2, 128, COUT)
        .transpose(1, 2, 0, 3)
        .reshape(2, 128, NTAP * COUT)
        .astype(_BF16)
    )
    wpT = (
        (np.asarray(inputs["wp"], f32)[:, :, 0, 0] * scales[4][:, None])
        .T.astype(_BF16)
        .copy()
    )
    return wb, wpT, bias_arr


def host_prep_x(x, padded=False):
    # transpose each image to (col, row) matching the device's column-major
    # SBUF layout, so the device DMA is one contiguous copy per cin-chunk.
    # padded=True bakes the 18-row zero pad (sim variant only).
    x = np.asarray(x, np.float32).reshape(B, 2, 128, H, H)
    if padded:
        xt = np.zeros((B, 2, 128, H, XR), np.float32)
        xt[:, :, :, :, PAD : PAD + H] = x.transpose(0, 1, 2, 4, 3)
        return xt.reshape(B, 2, 128, H * XR).astype(_BF16)
    xt = np.zeros((B, 2, 128, H, 66), np.float32)
    xt[:, :, :, :, :H] = x.transpose(0, 1, 2, 4, 3)
    return xt.reshape(B, 2, 128, H * 66).astype(_BF16)


def make_in_maps(inputs, padded=False):
    wb, wpT, bias_arr = host_prep_weights(inputs)
    xq = host_prep_x(inputs["x"], padded=padded)
    return [{"xp": xq[b], "wb": wb, "wp": wpT, "bias": bias_arr} for b in range(B)]


_NC_CACHE = []


def kernel(**inputs):
    from concourse import bass_utils

    if not _NC_CACHE:
        _NC_CACHE.append(build_program())
    nc = _NC_CACHE[0]
    in_maps = make_in_maps(inputs)
    res = bass_utils.run_bass_kernel_spmd(nc, in_maps, core_ids=list(range(N_CORES)))
    return np.stack(
        [r["out"].reshape(COUT, 2 * H, 2 * H) for r in res.results]
    ).astype(np.float32)
